# revision 27
# baseline (speedup 1.0000x reference)
"""Trainium2 Bass kernel for nn_Attention_79207786873625.

Non-local attention block: 1x1 convs (theta/phi/g) -> maxpool2x2(phi,g) ->
scores = theta^T phi -> softmax over m -> o = g beta^T -> w_o conv ->
gamma*o + x.   Shapes: B=16, C=256, H=W=64 (n=HW=4096, m=HW/4=1024).

Sharding: data-parallel over batch across 8 cores (2 samples/core),
weights replicated, per-sample score matrix device-local.

Design (v2):
- Scores are computed TRANSPOSED, sT[m, n] (m on partitions), so the
  attend contraction (over m) needs no transposition of the big matrix;
  only g (tiny) is PE-transposed. Softmax max-subtraction is skipped
  (|scores| ~ 30, exp stays comfortably in fp32 range).
- exp(sT) is written in BF16: attend matmuls run bf16 (same PE rate as
  fp32r, half the SBUF traffic) and the softmax-denominator partials can
  be pre-reduced on DVE at its 2x bf16 rate.
- Softmax denominator: instead of 8 PE ones-matmuls per (q,sub) (one per
  m-tile), DVE pre-merges the 8 exp tiles down to 3 (adds a=t01, b=t23,
  c=t45, d=t67, e=a+b) and the PE streams only {e,c,d}: 3 ones-matmuls.
  Cuts the PE denominator cost from 32768 to 12288 cols/sample.
- Engine rebalance: pooling maxes read conv PSUM directly on the Pool
  engine (kills the g_full ACT copy); theta copy on ACT is [32,1024]
  only; output stt is split h=0 on DVE / h=1 on Pool.
- All fp32 matmuls run float32r (TF32-like, 1 cycle/row vs 4 for fp32).
- Software pipelining: iteration q emits scores+exp for quarter q
  interleaved per-m-tile with attend/denom/w_o/output for quarter q-1,
  so ACT exp hides under PE attend. The next sample's convs+pools are
  emitted inside the current sample's last (attend-only) iteration,
  and its x is prefetched mid-loop on the ACT DMA ring.
- PSUM plan: tag "sT" [128,2,512] x2 bufs + tag "oud" [128,2,512] x2
  bufs = exactly 8 banks; w_o output reuses the freed oud tile.
"""
import sys

sys.path.insert(0, '/opt/trn_rl_repo')

from contextlib import ExitStack

import numpy as np

import concourse.bass as bass
import concourse.tile as tile
from concourse import bacc, mybir
from concourse.bass_utils import run_bass_kernel_spmd
from concourse.masks import make_identity

F32 = mybir.dt.float32
F32R = mybir.dt.float32r
BF16 = mybir.dt.bfloat16
AF = mybir.ActivationFunctionType
OP = mybir.AluOpType

B, C, H, W = 16, 256, 64, 64
HW = H * W            # 4096
M_POOL = HW // 4      # 1024
NCORES = 8
BPC = B // NCORES     # samples per core = 2

# Where (q, u) in the main loop to emit the NEXT sample's conv chunks.
# All in the attend-only q=4 window (sharing the idle "sT" PSUM tag), but
# at u=1,2 / 5,6 so each chunk's DVE pooling chain is queued ahead of the
# seg==3 output-stt bursts at u=3 / u=7.
CONV_SCHED = {(4, 1): 0, (4, 3): 1, (4, 5): 2, (4, 7): 3}


def build_kernel(nc, tc, ctx, x_d, wt_d, wp_d, wg_d, wo_d, gamma_d, out_d):
    sb = ctx.enter_context(tc.tile_pool(name="sb", bufs=1))
    per_s = ctx.enter_context(tc.tile_pool(name="per_s", bufs=2))
    stage1 = ctx.enter_context(tc.tile_pool(name="stage1", bufs=1))
    expp = ctx.enter_context(tc.tile_pool(name="expp", bufs=2))
    outp = ctx.enter_context(tc.tile_pool(name="outp", bufs=2))
    xp = ctx.enter_context(tc.tile_pool(name="xp", bufs=1))
    big = ctx.enter_context(tc.tile_pool(name="big", bufs=1, space="PSUM"))

    def load_x(b, eng, start=0):
        qs = []
        for qq in range(start, 4):
            x_t = xp.tile([128, 2, 1024], F32R, name="x_t", bufs=5)
            eng.dma_start(
                x_t[:],
                x_d[b].rearrange("(c2 p) n -> p c2 n", p=128)[:, :, 1024 * qq:1024 * qq + 1024].bitcast(F32R),
            )
            qs.append(x_t)
        return qs

    # ---- constants (ordered to unblock convs ASAP) ----
    ident_f = sb.tile([128, 128], F32)
    make_identity(nc, ident_f[:])
    ident = sb.tile([128, 128], F32R)
    nc.vector.tensor_copy(ident[:], ident_f[:])

    wtp_nat = sb.tile([64, 256], F32R)
    nc.sync.dma_start(wtp_nat[0:32, :], wt_d.bitcast(F32R))
    nc.sync.dma_start(wtp_nat[32:64, :], wp_d.bitcast(F32R))
    x_q0 = xp.tile([128, 2, 1024], F32R, name="x_t", bufs=5)
    for _hx in range(2):
        nc.sync.dma_start(
            x_q0[:, :, 512 * _hx:512 * _hx + 512],
            x_d[0].rearrange("(c2 p) n -> p c2 n", p=128)[:, :, 512 * _hx:512 * _hx + 512].bitcast(F32R),
        )
    wg_nat = sb.tile([128, 256], F32R)
    nc.sync.dma_start(wg_nat[:], wg_d.bitcast(F32R))

    x_qs_next = [x_q0] + load_x(0, nc.sync, start=1)

    wo_nat = sb.tile([128, 2, 128], F32R)
    nc.sync.dma_start(
        wo_nat[:], wo_d.rearrange("(two p) c -> p two c", p=128).bitcast(F32R)
    )
    gamma_bc = sb.tile([128, 1], F32)
    nc.sync.dma_start(gamma_bc[:], gamma_d.to_broadcast((128, 1)))

    ident_bf = sb.tile([128, 128], BF16)
    nc.vector.tensor_copy(ident_bf[:], ident_f[:])

    wtp = sb.tile([128, 2, 64], F32R)     # [c_in_chunk, chunk, 64=theta|phi]
    wg = sb.tile([128, 2, 128], F32R)     # [c_in_chunk, chunk, 128 g-ch]
    for cc in range(2):
        trp_ps = big.tile([128, 2, 512], F32R, name="trp_ps", tag="oud", bufs=2)
        nc.tensor.transpose(
            trp_ps[:, 0, 0:64], wtp_nat[:, 128 * cc:128 * cc + 128], ident[0:64, 0:64]
        )
        nc.vector.tensor_copy(wtp[:, cc, :], trp_ps[:, 0, 0:64])
    for cc in range(2):
        trw_ps = big.tile([128, 2, 512], F32R, name="trw_ps", tag="oud", bufs=2)
        nc.tensor.transpose(
            trw_ps[:, 0, 0:128], wg_nat[:, 128 * cc:128 * cc + 128], ident[:]
        )
        nc.vector.tensor_copy(wg[:, cc, :], trw_ps[:, 0, 0:128])

    ones_f = sb.tile([128, 128], F32)
    nc.vector.memset(ones_f[:], 1.0)
    ones_bf = sb.tile([128, 128], BF16)
    nc.vector.tensor_copy(ones_bf[:], ones_f[:])

    woT = sb.tile([128, 2, 128], F32R)    # [c(128), half, oc(128)]
    wo_pending = [True]

    def emit_wo_transposes():
        if not wo_pending[0]:
            return
        wo_pending[0] = False
        for cc in range(2):
            trg_ps = big.tile([128, 2, 512], F32R, name="trg_ps", tag="oud", bufs=2)
            nc.tensor.transpose(trg_ps[:, 0, 0:128], wo_nat[:, cc, :], ident[:])
            nc.vector.tensor_copy(woT[:, cc, :], trg_ps[:, 0, 0:128])

    def make_state():
        st = {}
        st["theta_q"] = []
        phi = per_s.tile([32, M_POOL], F32R, name="phi")
        g_sb = per_s.tile([128, M_POOL], BF16, name="g_sb")
        gT = per_s.tile([128, 8, 128], BF16, name="gT", bufs=1)
        phi1 = stage1.tile([32, 64, 32], F32, name="phi1")
        g1 = stage1.tile([128, 64, 32], F32, name="g1")
        st["phi"], st["g_sb"], st["gT"], st["phi1"], st["g1"] = phi, g_sb, gT, phi1, g1
        return st

    def emit_tree_adds(expST, ts, u):
        # bf16 pair-merge tree for the softmax denominator:
        # a=t01, b=t23, c=t45, d=t67, e=a+b; PE streams {e,c,d}.
        # a/b/e go to the otherwise-idle Pool engine, c/d to DVE.
        if u == 1:
            ts["a"] = expp.tile([128, 1024], BF16, name="ts_a", bufs=2)
            nc.gpsimd.tensor_tensor(
                ts["a"][:], expST[:, 0, :], expST[:, 1, :], op=OP.add)
        elif u == 3:
            ts["b"] = expp.tile([128, 1024], BF16, name="ts_b", bufs=2)
            nc.gpsimd.tensor_tensor(
                ts["b"][:], expST[:, 2, :], expST[:, 3, :], op=OP.add)
            ts["e"] = expp.tile([128, 1024], BF16, name="ts_e", bufs=2)
            nc.gpsimd.tensor_tensor(
                ts["e"][:], ts["a"][:], ts["b"][:], op=OP.add)
        elif u == 5:
            ts["c"] = expp.tile([128, 1024], BF16, name="ts_c", bufs=2)
            nc.vector.tensor_tensor(
                ts["c"][:], expST[:, 4, :], expST[:, 5, :], op=OP.add)
        elif u == 7:
            ts["d"] = expp.tile([128, 1024], BF16, name="ts_d", bufs=2)
            nc.vector.tensor_tensor(
                ts["d"][:], expST[:, 6, :], expST[:, 7, :], op=OP.add)

    def emit_q0_unit(st, u):
        # scores + exp + denominator tree adds for quarter 0, emitted right
        # after the conv chunk that produced phi m-chunk u//2 (hides the
        # quarter-0 work under the conv/x-DMA pipeline).
        if "expST0" not in st:
            st["expST0"] = expp.tile([128, 8, 1024], BF16, name="expST")
            st["ts0"] = {}
        expST, ts = st["expST0"], st["ts0"]
        sT_ps = big.tile([128, 2, 512], F32, name="sT_ps", tag="sT", bufs=2)
        for sub in range(2):
            nc.tensor.matmul(
                sT_ps[:, sub, :],
                st["phi"][:, 128 * u:128 * u + 128],
                st["theta_q"][0][0:32, 512 * sub:512 * sub + 512],
                start=True, stop=True,
            )
        nc.scalar.activation(expST[:, u, :], sT_ps[:, :, :], AF.Exp)
        emit_tree_adds(expST, ts, u)

    def emit_gtr(st, c4):
        g_sb, gT = st["g_sb"], st["gT"]
        for t in (2 * c4, 2 * c4 + 1):
            gtr_ps = big.tile([128, 2, 512], F32, name="gtr_ps", tag="oud", bufs=2)
            bview = gtr_ps[:, 0, 0:64].bitcast(BF16)  # [128, 128] bf16
            nc.tensor.transpose(
                bview, g_sb[:, 128 * t:128 * t + 128], ident_bf[:]
            )
            nc.scalar.copy(gT[:, t, :], bview)

    def emit_conv_chunk(st, x_qs_b, c4, g_tag, do_gtr=True):
        phi, g_sb, gT, phi1, g1 = st["phi"], st["g_sb"], st["gT"], st["phi1"], st["g1"]
        tp_ps = big.tile([64, 2, 512], F32, name="tp_ps", tag="sT", bufs=2)
        g_ps = big.tile([128, 2, 512], F32, name="g_ps", tag=g_tag, bufs=2)
        for sub in range(2):
            xoff = 512 * sub
            nc.tensor.matmul(
                tp_ps[:, sub, :], wtp[:, 0, :], x_qs_b[c4][:, 0, xoff:xoff + 512],
                start=True, stop=False,
            )
            nc.tensor.matmul(
                tp_ps[:, sub, :], wtp[:, 1, :], x_qs_b[c4][:, 1, xoff:xoff + 512],
                start=False, stop=True,
            )
        for sub in range(2):
            xoff = 512 * sub
            nc.tensor.matmul(
                g_ps[:, sub, :], wg[:, 0, :], x_qs_b[c4][:, 0, xoff:xoff + 512],
                start=True, stop=False,
            )
            nc.tensor.matmul(
                g_ps[:, sub, :], wg[:, 1, :], x_qs_b[c4][:, 1, xoff:xoff + 512],
                start=False, stop=True,
            )
        # theta+phi PSUM->SBUF copy (ACT); g pooling reads PSUM on DVE
        # (GPSIMD cannot access PSUM), so the g_full copy is eliminated.
        tpf = stage1.tile([64, 1024], F32R, name="tpf", bufs=4)
        nc.scalar.copy(tpf[:], tp_ps[:, :, :])
        st["theta_q"].append(tpf)
        pfv = tpf[32:64, :].bitcast(F32).rearrange("p (h w2 t) -> p h w2 t", w2=32, t=2)
        nc.vector.tensor_tensor(
            phi1[:, 16 * c4:16 * c4 + 16, :],
            pfv[:, :, :, 0], pfv[:, :, :, 1], op=OP.max,
        )
        gfv = g_ps[:, :, :].rearrange("p a (h w2 t) -> p (a h) w2 t", w2=32, t=2)
        nc.vector.tensor_reduce(
            g1[:, 16 * c4:16 * c4 + 16, :], gfv,
            axis=mybir.AxisListType.X, op=OP.max,
        )
        # pool step 2 (rows) for this chunk
        p1v = phi1[:, 16 * c4:16 * c4 + 16, :].rearrange("p (i t) w -> p i t w", t=2)
        nc.vector.tensor_tensor(
            phi[:, 256 * c4:256 * c4 + 256].rearrange("p (i w) -> p i w", w=32),
            p1v[:, :, 0, :], p1v[:, :, 1, :], op=OP.max,
        )
        g1v = g1[:, 16 * c4:16 * c4 + 16, :].rearrange("p (i t) w -> p i t w", t=2)
        nc.vector.tensor_tensor(
            g_sb[:, 256 * c4:256 * c4 + 256].rearrange("p (i w) -> p i w", w=32),
            g1v[:, :, 0, :], g1v[:, :, 1, :], op=OP.max,
        )
        if do_gtr:
            emit_gtr(st, c4)

    st = make_state()
    for c4 in range(4):
        emit_conv_chunk(st, x_qs_next, c4, "oud")
    st_next = None
    for b in range(BPC):
        x_qs = x_qs_next
        theta_q, phi, gT = st["theta_q"], st["phi"], st["gT"]
        emit_wo_transposes()
        # ---- main loop over n-quarters, software pipelined ----
        # For sample 0, quarter 0's scores/exp were emitted with the convs
        # (emit_q0_unit) so the loop starts at q=1; iteration q: scores+exp
        # for quarter q (q<4) interleaved per m-tile with attend/denom/wo/out
        # for quarter q-1 (q>=1).
        if "expST0" in st:
            expST_prev, ts_prev, q_start = st["expST0"], st["ts0"], 1
        else:
            expST_prev, ts_prev, q_start = None, None, 0
        last = b == BPC - 1
        for q in range(q_start, 5):
            expST = None
            ts = {}
            if q < 4:
                expST = expp.tile([128, 8, 1024], BF16, name="expST")
            out_q = None
            oud = {}
            if q == 2 and b + 1 < BPC:
                x_qs_next = load_x(b + 1, nc.scalar)
            for u in range(8):  # per m-tile unit
                if q < 4:
                    sT_ps = big.tile([128, 2, 512], F32, name="sT_ps", tag="sT", bufs=2)
                    for sub in range(2):
                        nc.tensor.matmul(
                            sT_ps[:, sub, :],
                            phi[:, 128 * u:128 * u + 128],
                            theta_q[q][0:32, 512 * sub:512 * sub + 512],
                            start=True, stop=True,
                        )
                    nc.scalar.activation(
                        expST[:, u, :], sT_ps[:, :, :], AF.Exp
                    )
                    emit_tree_adds(expST, ts, u)
                sub, seg = u // 4, u % 4  # seg: 2 m-tiles each
                if q >= 1 and seg == 0:
                    oud[sub] = big.tile([128, 2, 512], F32, name="oud", tag="oud", bufs=2)
                if q >= 1:
                    for tl in range(2):
                        t = 2 * seg + tl
                        nc.tensor.matmul(
                            oud[sub][:, 0, :], gT[:, t, :],
                            expST_prev[:, t, 512 * sub:512 * sub + 512],
                            start=(t == 0), stop=(t == 7),
                        )
                if q >= 1 and seg == 3:
                    # softmax denominator: 3 ones-matmuls over {e,c,d}
                    for i, key in enumerate(("e", "c", "d")):
                        nc.tensor.matmul(
                            oud[sub][:, 1, :], ones_bf[:],
                            ts_prev[key][:, 512 * sub:512 * sub + 512],
                            start=(i == 0), stop=(i == 2),
                        )
                    if out_q is None:
                        out_q = outp.tile([128, 2, 1024], F32, name="out_q")
                    xoff = 512 * sub
                    nqp = 1024 * (q - 1)
                    recip = outp.tile([128, 512], F32, name="recip")
                    oUr = outp.tile([128, 512], F32R, name="oUr")
                    nc.vector.reciprocal_approx_fast(
                        out=recip[:], in_=oud[sub][:, 1, :]
                    )
                    nc.vector.scalar_tensor_tensor(
                        oUr[:], oud[sub][:, 0, :], 1.0, recip[:],
                        op0=OP.mult, op1=OP.mult,
                    )
                    for h in range(2):
                        nc.tensor.matmul(
                            oud[sub][:, h, :], woT[:, h, :], oUr[:],
                            start=True, stop=True,
                        )
                    for h in range(2):
                        nc.vector.scalar_tensor_tensor(
                            out_q[:, h, xoff:xoff + 512],
                            oud[sub][:, h, :], gamma_bc[:],
                            x_qs[q - 1][:, h, xoff:xoff + 512].bitcast(F32),
                            op0=OP.mult, op1=OP.add,
                        )
                    if last and q == 4:
                        for h in range(2):
                            nc.sync.dma_start(
                                out_d[b, 128 * h:128 * h + 128,
                                      nqp + xoff:nqp + xoff + 512],
                                out_q[:, h, xoff:xoff + 512],
                            )
                conv_c4 = CONV_SCHED.get((q, u))
                if conv_c4 is not None and b + 1 < BPC:
                    if st_next is None:
                        st_next = make_state()
                    emit_conv_chunk(st_next, x_qs_next, conv_c4, "sT", do_gtr=False)
            if q >= 1 and not (last and q == 4):
                nqp = 1024 * (q - 1)
                for h in range(2):
                    nc.sync.dma_start(
                        out_d[b, 128 * h:128 * h + 128, nqp:nqp + 1024],
                        out_q[:, h, :],
                    )
            expST_prev = expST
            ts_prev = ts if q < 4 else None
        if st_next is not None:
            for c4 in range(4):
                emit_gtr(st_next, c4)
        st = st_next
        st_next = None


_CACHE = {}


def _get_compiled():
    if "nc" in _CACHE:
        return _CACHE["nc"]
    nc = bacc.Bacc("TRN2", target_bir_lowering=False, debug=False,
                   num_devices=NCORES)
    x_d = nc.dram_tensor("x", [BPC, C, HW], F32, kind="ExternalInput").ap()
    wt_d = nc.dram_tensor("w_theta", [32, 256], F32, kind="ExternalInput").ap()
    wp_d = nc.dram_tensor("w_phi", [32, 256], F32, kind="ExternalInput").ap()
    wg_d = nc.dram_tensor("w_g", [128, 256], F32, kind="ExternalInput").ap()
    wo_d = nc.dram_tensor("w_o", [256, 128], F32, kind="ExternalInput").ap()
    gamma_d = nc.dram_tensor("gamma", [1, 1], F32, kind="ExternalInput").ap()
    out_d = nc.dram_tensor("out", [BPC, C, HW], F32, kind="ExternalOutput").ap()

    with tile.TileContext(nc) as tc:
        with ExitStack() as ctx:
            build_kernel(nc, tc, ctx, x_d, wt_d, wp_d, wg_d, wo_d, gamma_d,
                         out_d)
    nc.compile()
    _CACHE["nc"] = nc
    return nc


def kernel(x, w_theta, w_phi, w_g, w_o, gamma, _trace=False, _tmpdir=None):
    nc = _get_compiled()
    x = np.ascontiguousarray(np.asarray(x, dtype=np.float32))
    in_maps = []
    for c in range(NCORES):
        shard = x[c * BPC:(c + 1) * BPC].reshape(BPC, C, HW)
        in_maps.append({
            "x": np.ascontiguousarray(shard),
            "w_theta": np.asarray(w_theta, np.float32),
            "w_phi": np.asarray(w_phi, np.float32),
            "w_g": np.asarray(w_g, np.float32),
            "w_o": np.asarray(w_o, np.float32),
            "gamma": np.asarray(gamma, np.float32).reshape(1, 1),
        })
    kwargs = {}
    if _trace:
        kwargs = dict(trace=True, tmpdir=_tmpdir)
    res = run_bass_kernel_spmd(nc, in_maps, core_ids=list(range(NCORES)),
                               **kwargs)
    out = np.concatenate([r["out"] for r in res.results], axis=0)
    out = out.reshape(B, C, H, W).astype(np.float32)
    if _trace:
        return out, res
    return out


# revision 33
# speedup vs baseline: 1.0527x; 1.0527x over previous
"""Trainium2 Bass kernel for nn_Attention_79207786873625.

Non-local attention block: 1x1 convs (theta/phi/g) -> maxpool2x2(phi,g) ->
scores = theta^T phi -> softmax over m -> o = g beta^T -> w_o conv ->
gamma*o + x.   Shapes: B=16, C=256, H=W=64 (n=HW=4096, m=HW/4=1024).

Sharding: data-parallel over batch across 8 cores (2 samples/core),
weights replicated, per-sample score matrix device-local.

Design (v2):
- Scores are computed TRANSPOSED, sT[m, n] (m on partitions), so the
  attend contraction (over m) needs no transposition of the big matrix;
  only g (tiny) is PE-transposed. Softmax max-subtraction is skipped
  (|scores| ~ 30, exp stays comfortably in fp32 range).
- exp(sT) is written in BF16: attend matmuls run bf16 (same PE rate as
  fp32r, half the SBUF traffic) and the softmax-denominator partials can
  be pre-reduced on DVE at its 2x bf16 rate.
- Softmax denominator: instead of 8 PE ones-matmuls per (q,sub) (one per
  m-tile), DVE pre-merges the 8 exp tiles down to 3 (adds a=t01, b=t23,
  c=t45, d=t67, e=a+b) and the PE streams only {e,c,d}: 3 ones-matmuls.
  Cuts the PE denominator cost from 32768 to 12288 cols/sample.
- Engine rebalance: pooling maxes read conv PSUM directly on the Pool
  engine (kills the g_full ACT copy); theta copy on ACT is [32,1024]
  only; output stt is split h=0 on DVE / h=1 on Pool.
- All fp32 matmuls run float32r (TF32-like, 1 cycle/row vs 4 for fp32).
- Software pipelining: iteration q emits scores+exp for quarter q
  interleaved per-m-tile with attend/denom/w_o/output for quarter q-1,
  so ACT exp hides under PE attend. The next sample's convs+pools are
  emitted inside the current sample's last (attend-only) iteration,
  and its x is prefetched mid-loop on the ACT DMA ring.
- PSUM plan: tag "sT" [128,2,512] x2 bufs + tag "oud" [128,2,512] x2
  bufs = exactly 8 banks; w_o output reuses the freed oud tile.
"""
import sys

sys.path.insert(0, '/opt/trn_rl_repo')

from contextlib import ExitStack

import numpy as np

import concourse.bass as bass
import concourse.tile as tile
from concourse import bacc, mybir
from concourse.bass_utils import run_bass_kernel_spmd
from concourse.masks import make_identity

F32 = mybir.dt.float32
F32R = mybir.dt.float32r
BF16 = mybir.dt.bfloat16
AF = mybir.ActivationFunctionType
OP = mybir.AluOpType

B, C, H, W = 16, 256, 64, 64
HW = H * W            # 4096
M_POOL = HW // 4      # 1024
NCORES = 8
BPC = B // NCORES     # samples per core = 2

# Where (q, u) in the main loop to emit the NEXT sample's conv chunks.
# All in the attend-only q=4 window (sharing the idle "sT" PSUM tag), but
# at u=1,2 / 5,6 so each chunk's DVE pooling chain is queued ahead of the
# seg==3 output-stt bursts at u=3 / u=7.
CONV_SCHED = {(4, 1): 0, (4, 3): 1, (4, 5): 2, (4, 7): 3}


def build_kernel(nc, tc, ctx, x_d, wt_d, wp_d, wg_d, wo_d, gamma_d, out_d):
    sb = ctx.enter_context(tc.tile_pool(name="sb", bufs=1))
    per_s = ctx.enter_context(tc.tile_pool(name="per_s", bufs=2))
    stage1 = ctx.enter_context(tc.tile_pool(name="stage1", bufs=1))
    expp = ctx.enter_context(tc.tile_pool(name="expp", bufs=2))
    outp = ctx.enter_context(tc.tile_pool(name="outp", bufs=2))
    xp = ctx.enter_context(tc.tile_pool(name="xp", bufs=1))
    big = ctx.enter_context(tc.tile_pool(name="big", bufs=1, space="PSUM"))

    def load_x_quarter(b, qq, eng):
        x_t = xp.tile([128, 2, 1024], F32R, name="x_t", bufs=5)
        eng.dma_start(
            x_t[:],
            x_d[b].rearrange("(c2 p) n -> p c2 n", p=128)[:, :, 1024 * qq:1024 * qq + 1024].bitcast(F32R),
        )
        return x_t

    def load_x(b, eng, start=0):
        return [load_x_quarter(b, qq, eng) for qq in range(start, 4)]

    # ---- constants (ordered to unblock convs ASAP) ----
    ident_f = sb.tile([128, 128], F32)
    make_identity(nc, ident_f[:])
    ident = sb.tile([128, 128], F32R)
    nc.vector.tensor_copy(ident[:], ident_f[:])

    wtp_nat = sb.tile([64, 256], F32R)
    nc.sync.dma_start(wtp_nat[0:32, :], wt_d.bitcast(F32R))
    nc.sync.dma_start(wtp_nat[32:64, :], wp_d.bitcast(F32R))
    x_q0 = xp.tile([128, 2, 1024], F32R, name="x_t", bufs=5)
    for _hx in range(2):
        nc.sync.dma_start(
            x_q0[:, :, 512 * _hx:512 * _hx + 512],
            x_d[0].rearrange("(c2 p) n -> p c2 n", p=128)[:, :, 512 * _hx:512 * _hx + 512].bitcast(F32R),
        )
    wg_nat = sb.tile([128, 256], F32R)
    nc.sync.dma_start(wg_nat[:], wg_d.bitcast(F32R))

    x_qs_next = [x_q0] + load_x(0, nc.sync, start=1)

    wo_nat = sb.tile([128, 2, 128], F32R)
    nc.sync.dma_start(
        wo_nat[:], wo_d.rearrange("(two p) c -> p two c", p=128).bitcast(F32R)
    )
    gamma_bc = sb.tile([128, 1], F32)
    nc.sync.dma_start(gamma_bc[:], gamma_d.to_broadcast((128, 1)))

    ident_bf = sb.tile([128, 128], BF16)
    nc.vector.tensor_copy(ident_bf[:], ident_f[:])

    wtp = sb.tile([128, 2, 64], F32R)     # [c_in_chunk, chunk, 64=theta|phi]
    wg = sb.tile([128, 2, 128], F32R)     # [c_in_chunk, chunk, 128 g-ch]
    for cc in range(2):
        trp_ps = big.tile([128, 2, 512], F32R, name="trp_ps", tag="oud", bufs=2)
        nc.tensor.transpose(
            trp_ps[:, 0, 0:64], wtp_nat[:, 128 * cc:128 * cc + 128], ident[0:64, 0:64]
        )
        nc.vector.tensor_copy(wtp[:, cc, :], trp_ps[:, 0, 0:64])
    for cc in range(2):
        trw_ps = big.tile([128, 2, 512], F32R, name="trw_ps", tag="oud", bufs=2)
        nc.tensor.transpose(
            trw_ps[:, 0, 0:128], wg_nat[:, 128 * cc:128 * cc + 128], ident[:]
        )
        nc.vector.tensor_copy(wg[:, cc, :], trw_ps[:, 0, 0:128])

    ones_f = sb.tile([128, 128], F32)
    nc.vector.memset(ones_f[:], 1.0)
    ones_bf = sb.tile([128, 128], BF16)
    nc.vector.tensor_copy(ones_bf[:], ones_f[:])

    woT = sb.tile([128, 2, 128], F32R)    # [c(128), half, oc(128)]
    wo_pending = [True]

    def emit_wo_transposes():
        if not wo_pending[0]:
            return
        wo_pending[0] = False
        for cc in range(2):
            trg_ps = big.tile([128, 2, 512], F32R, name="trg_ps", tag="oud", bufs=2)
            nc.tensor.transpose(trg_ps[:, 0, 0:128], wo_nat[:, cc, :], ident[:])
            nc.vector.tensor_copy(woT[:, cc, :], trg_ps[:, 0, 0:128])

    def make_state():
        st = {}
        st["theta_q"] = []
        phi = per_s.tile([32, M_POOL], F32R, name="phi")
        g_sb = per_s.tile([128, M_POOL], BF16, name="g_sb")
        gT = per_s.tile([128, 8, 128], BF16, name="gT", bufs=1)
        phi1 = stage1.tile([32, 64, 32], F32, name="phi1")
        g1 = stage1.tile([128, 64, 32], F32, name="g1")
        st["phi"], st["g_sb"], st["gT"], st["phi1"], st["g1"] = phi, g_sb, gT, phi1, g1
        return st

    def emit_tree_adds(expST, ts, u):
        # bf16 pair-merge tree for the softmax denominator:
        # a=t01, b=t23, c=t45, d=t67, e=a+b; PE streams {e,c,d}.
        # a/b/e go to the otherwise-idle Pool engine, c/d to DVE.
        if u == 1:
            ts["a"] = expp.tile([128, 1024], BF16, name="ts_a", bufs=2)
            nc.gpsimd.tensor_tensor(
                ts["a"][:], expST[:, 0, :], expST[:, 1, :], op=OP.add)
        elif u == 3:
            ts["b"] = expp.tile([128, 1024], BF16, name="ts_b", bufs=2)
            nc.gpsimd.tensor_tensor(
                ts["b"][:], expST[:, 2, :], expST[:, 3, :], op=OP.add)
            ts["e"] = expp.tile([128, 1024], BF16, name="ts_e", bufs=2)
            nc.gpsimd.tensor_tensor(
                ts["e"][:], ts["a"][:], ts["b"][:], op=OP.add)
        elif u == 5:
            ts["c"] = expp.tile([128, 1024], BF16, name="ts_c", bufs=2)
            nc.vector.tensor_tensor(
                ts["c"][:], expST[:, 4, :], expST[:, 5, :], op=OP.add)
        elif u == 7:
            ts["d"] = expp.tile([128, 1024], BF16, name="ts_d", bufs=2)
            nc.vector.tensor_tensor(
                ts["d"][:], expST[:, 6, :], expST[:, 7, :], op=OP.add)

    def emit_q0_unit(st, u):
        # scores + exp + denominator tree adds for quarter 0, emitted right
        # after the conv chunk that produced phi m-chunk u//2 (hides the
        # quarter-0 work under the conv/x-DMA pipeline).
        if "expST0" not in st:
            st["expST0"] = expp.tile([128, 8, 1024], BF16, name="expST")
            st["ts0"] = {}
        expST, ts = st["expST0"], st["ts0"]
        sT_ps = big.tile([128, 2, 512], F32, name="sT_ps", tag="sT", bufs=2)
        for sub in range(2):
            nc.tensor.matmul(
                sT_ps[:, sub, :],
                st["phi"][:, 128 * u:128 * u + 128],
                st["theta_q"][0][0:32, 512 * sub:512 * sub + 512],
                start=True, stop=True,
            )
        nc.scalar.activation(expST[:, u, :], sT_ps[:, :, :], AF.Exp)
        emit_tree_adds(expST, ts, u)

    def emit_gtr(st, c4):
        g_sb, gT = st["g_sb"], st["gT"]
        for t in (2 * c4, 2 * c4 + 1):
            gtr_ps = big.tile([128, 2, 512], F32, name="gtr_ps", tag="oud", bufs=2)
            bview = gtr_ps[:, 0, 0:64].bitcast(BF16)  # [128, 128] bf16
            nc.tensor.transpose(
                bview, g_sb[:, 128 * t:128 * t + 128], ident_bf[:]
            )
            nc.vector.tensor_copy(gT[:, t, :], bview)

    def emit_conv_chunk(st, x_qs_b, c4, g_tag, do_gtr=True):
        phi, g_sb, gT, phi1, g1 = st["phi"], st["g_sb"], st["gT"], st["phi1"], st["g1"]
        tp_ps = big.tile([64, 2, 512], F32, name="tp_ps", tag="sT", bufs=2)
        g_ps = big.tile([128, 2, 512], F32, name="g_ps", tag=g_tag, bufs=2)
        for sub in range(2):
            xoff = 512 * sub
            nc.tensor.matmul(
                tp_ps[:, sub, :], wtp[:, 0, :], x_qs_b[c4][:, 0, xoff:xoff + 512],
                start=True, stop=False,
            )
            nc.tensor.matmul(
                tp_ps[:, sub, :], wtp[:, 1, :], x_qs_b[c4][:, 1, xoff:xoff + 512],
                start=False, stop=True,
            )
        for sub in range(2):
            xoff = 512 * sub
            nc.tensor.matmul(
                g_ps[:, sub, :], wg[:, 0, :], x_qs_b[c4][:, 0, xoff:xoff + 512],
                start=True, stop=False,
            )
            nc.tensor.matmul(
                g_ps[:, sub, :], wg[:, 1, :], x_qs_b[c4][:, 1, xoff:xoff + 512],
                start=False, stop=True,
            )
        # theta+phi PSUM->SBUF copy (ACT); g pooling reads PSUM on DVE
        # (GPSIMD cannot access PSUM), so the g_full copy is eliminated.
        tpf = stage1.tile([64, 1024], F32R, name="tpf", bufs=4)
        nc.scalar.copy(tpf[:], tp_ps[:, :, :])
        st["theta_q"].append(tpf)
        pfv = tpf[32:64, :].bitcast(F32).rearrange("p (h w2 t) -> p h w2 t", w2=32, t=2)
        nc.vector.tensor_tensor(
            phi1[:, 16 * c4:16 * c4 + 16, :],
            pfv[:, :, :, 0], pfv[:, :, :, 1], op=OP.max,
        )
        gfv = g_ps[:, :, :].rearrange("p a (h w2 t) -> p (a h) w2 t", w2=32, t=2)
        nc.vector.tensor_reduce(
            g1[:, 16 * c4:16 * c4 + 16, :], gfv,
            axis=mybir.AxisListType.X, op=OP.max,
        )
        # pool step 2 (rows) for this chunk
        p1v = phi1[:, 16 * c4:16 * c4 + 16, :].rearrange("p (i t) w -> p i t w", t=2)
        nc.vector.tensor_tensor(
            phi[:, 256 * c4:256 * c4 + 256].rearrange("p (i w) -> p i w", w=32),
            p1v[:, :, 0, :], p1v[:, :, 1, :], op=OP.max,
        )
        g1v = g1[:, 16 * c4:16 * c4 + 16, :].rearrange("p (i t) w -> p i t w", t=2)
        nc.vector.tensor_tensor(
            g_sb[:, 256 * c4:256 * c4 + 256].rearrange("p (i w) -> p i w", w=32),
            g1v[:, :, 0, :], g1v[:, :, 1, :], op=OP.max,
        )
        if do_gtr:
            emit_gtr(st, c4)

    st = make_state()
    for c4 in range(4):
        emit_conv_chunk(st, x_qs_next, c4, "oud")
    st_next = None
    for b in range(BPC):
        x_qs = x_qs_next
        x_qs_next = []
        theta_q, phi, gT = st["theta_q"], st["phi"], st["gT"]
        emit_wo_transposes()
        # ---- main loop over n-quarters, software pipelined ----
        # For sample 0, quarter 0's scores/exp were emitted with the convs
        # (emit_q0_unit) so the loop starts at q=1; iteration q: scores+exp
        # for quarter q (q<4) interleaved per m-tile with attend/denom/wo/out
        # for quarter q-1 (q>=1).
        if "expST0" in st:
            expST_prev, ts_prev, q_start = st["expST0"], st["ts0"], 1
        else:
            expST_prev, ts_prev, q_start = None, None, 0
        last = b == BPC - 1
        for q in range(q_start, 5):
            expST = None
            ts = {}
            if q < 4:
                expST = expp.tile([128, 8, 1024], BF16, name="expST")
            out_q = None
            oud = {}
            # Prefetch next sample's x one quarter per iteration on the sync
            # ring; each DMA's x-tile rotation wait is already clear when
            # issued, so the ring never stalls. Quarter 3 must go at the top
            # of q=4 (the conv for it runs mid-iteration).
            if q == 4 and b + 1 < BPC:
                x_qs_next.append(load_x_quarter(b + 1, 3, nc.sync))
            for u in range(8):  # per m-tile unit
                if q < 4:
                    sT_ps = big.tile([128, 2, 512], F32, name="sT_ps", tag="sT", bufs=2)
                    for sub in range(2):
                        nc.tensor.matmul(
                            sT_ps[:, sub, :],
                            phi[:, 128 * u:128 * u + 128],
                            theta_q[q][0:32, 512 * sub:512 * sub + 512],
                            start=True, stop=True,
                        )
                    nc.scalar.activation(
                        expST[:, u, :], sT_ps[:, :, :], AF.Exp
                    )
                    emit_tree_adds(expST, ts, u)
                sub, seg = u // 4, u % 4  # seg: 2 m-tiles each
                if q >= 1 and seg == 0:
                    oud[sub] = big.tile([128, 2, 512], F32, name="oud", tag="oud", bufs=2)
                if q >= 1:
                    for tl in range(2):
                        t = 2 * seg + tl
                        nc.tensor.matmul(
                            oud[sub][:, 0, :], gT[:, t, :],
                            expST_prev[:, t, 512 * sub:512 * sub + 512],
                            start=(t == 0), stop=(t == 7),
                        )
                if q >= 1 and seg == 3:
                    # softmax denominator: 3 ones-matmuls over {e,c,d}
                    for i, key in enumerate(("e", "c", "d")):
                        nc.tensor.matmul(
                            oud[sub][:, 1, :], ones_bf[:],
                            ts_prev[key][:, 512 * sub:512 * sub + 512],
                            start=(i == 0), stop=(i == 2),
                        )
                    if out_q is None:
                        out_q = outp.tile([128, 2, 1024], F32, name="out_q")
                    xoff = 512 * sub
                    nqp = 1024 * (q - 1)
                    recip = outp.tile([128, 512], F32, name="recip")
                    oUr = outp.tile([128, 512], F32R, name="oUr")
                    nc.vector.reciprocal_approx_fast(
                        out=recip[:], in_=oud[sub][:, 1, :]
                    )
                    nc.vector.scalar_tensor_tensor(
                        oUr[:], oud[sub][:, 0, :], 1.0, recip[:],
                        op0=OP.mult, op1=OP.mult,
                    )
                    for h in range(2):
                        nc.tensor.matmul(
                            oud[sub][:, h, :], woT[:, h, :], oUr[:],
                            start=True, stop=True,
                        )
                    for h in range(2):
                        nc.vector.scalar_tensor_tensor(
                            out_q[:, h, xoff:xoff + 512],
                            oud[sub][:, h, :], gamma_bc[:],
                            x_qs[q - 1][:, h, xoff:xoff + 512].bitcast(F32),
                            op0=OP.mult, op1=OP.add,
                        )
                    if last and q == 4:
                        for h in range(2):
                            nc.sync.dma_start(
                                out_d[b, 128 * h:128 * h + 128,
                                      nqp + xoff:nqp + xoff + 512],
                                out_q[:, h, xoff:xoff + 512],
                            )
                conv_c4 = CONV_SCHED.get((q, u))
                if conv_c4 is not None and b + 1 < BPC:
                    if st_next is None:
                        st_next = make_state()
                    emit_conv_chunk(st_next, x_qs_next, conv_c4, "sT", do_gtr=False)
            if q >= 1 and not (last and q == 4):
                nqp = 1024 * (q - 1)
                for h in range(2):
                    nc.sync.dma_start(
                        out_d[b, 128 * h:128 * h + 128, nqp:nqp + 1024],
                        out_q[:, h, :],
                    )
            if 1 <= q <= 3 and b + 1 < BPC:
                x_qs_next.append(load_x_quarter(b + 1, q - 1, nc.sync))
            expST_prev = expST
            ts_prev = ts if q < 4 else None
        if st_next is not None:
            for c4 in range(4):
                emit_gtr(st_next, c4)
        st = st_next
        st_next = None


_CACHE = {}


def _get_compiled():
    if "nc" in _CACHE:
        return _CACHE["nc"]
    nc = bacc.Bacc("TRN2", target_bir_lowering=False, debug=False,
                   num_devices=NCORES)
    x_d = nc.dram_tensor("x", [BPC, C, HW], F32, kind="ExternalInput").ap()
    wt_d = nc.dram_tensor("w_theta", [32, 256], F32, kind="ExternalInput").ap()
    wp_d = nc.dram_tensor("w_phi", [32, 256], F32, kind="ExternalInput").ap()
    wg_d = nc.dram_tensor("w_g", [128, 256], F32, kind="ExternalInput").ap()
    wo_d = nc.dram_tensor("w_o", [256, 128], F32, kind="ExternalInput").ap()
    gamma_d = nc.dram_tensor("gamma", [1, 1], F32, kind="ExternalInput").ap()
    out_d = nc.dram_tensor("out", [BPC, C, HW], F32, kind="ExternalOutput").ap()

    with tile.TileContext(nc) as tc:
        with ExitStack() as ctx:
            build_kernel(nc, tc, ctx, x_d, wt_d, wp_d, wg_d, wo_d, gamma_d,
                         out_d)
    nc.compile()
    _CACHE["nc"] = nc
    return nc


def kernel(x, w_theta, w_phi, w_g, w_o, gamma, _trace=False, _tmpdir=None):
    nc = _get_compiled()
    x = np.ascontiguousarray(np.asarray(x, dtype=np.float32))
    in_maps = []
    for c in range(NCORES):
        shard = x[c * BPC:(c + 1) * BPC].reshape(BPC, C, HW)
        in_maps.append({
            "x": np.ascontiguousarray(shard),
            "w_theta": np.asarray(w_theta, np.float32),
            "w_phi": np.asarray(w_phi, np.float32),
            "w_g": np.asarray(w_g, np.float32),
            "w_o": np.asarray(w_o, np.float32),
            "gamma": np.asarray(gamma, np.float32).reshape(1, 1),
        })
    kwargs = {}
    if _trace:
        kwargs = dict(trace=True, tmpdir=_tmpdir)
    res = run_bass_kernel_spmd(nc, in_maps, core_ids=list(range(NCORES)),
                               **kwargs)
    out = np.concatenate([r["out"] for r in res.results], axis=0)
    out = out.reshape(B, C, H, W).astype(np.float32)
    if _trace:
        return out, res
    return out
